# revision 27
# baseline (speedup 1.0000x reference)
"""DMN (Dynamic Memory Network) forward pass on 8 Trainium2 NeuronCores.

Fully data-parallel over batch (16 examples/core, no cross-core traffic).
Matmuls in bf16 with fp32 PSUM accumulation. GRU states live in a "folded
transposed" SBUF layout (128 partitions = one 128-row slice of H; free dim =
h_tile * batch + example), so the recurrent matmul's moving operand needs no
per-step transposes and pointwise ops stay wide.

kernel(**inputs) takes FULL unsharded inputs (as from reference.setup_inputs)
and returns the FULL (B*num_decode, V) fp32 output.
"""

import numpy as np
import ml_dtypes

import concourse.bacc as bacc
import concourse.mybir as mybir
import concourse.tile as tile
from concourse import bass_utils

F32 = mybir.dt.float32
BF16 = mybir.dt.bfloat16
FP8 = mybir.dt.float8e4
AF = mybir.ActivationFunctionType
ALU = mybir.AluOpType

H = 512
HQ = 4            # H / 128
G3 = 3 * H
MT = 12           # gate m-tiles
V = 32000
B = 128
NF = 40
L = 12
QL = 16
EPISODES = 3
N_CORES = 8
BC = B // N_CORES
FCHUNK = 320
VBLK = 512

bf16 = ml_dtypes.bfloat16
f8 = ml_dtypes.float8_e4m3

WS = 16.0          # fp8 weight scale; gates descale via activation scale
GSC = 1.0 / WS
FCW = 32.0         # fc weight scale (fp8)
FCS = 8.0 * FCW    # fc psum scale: hdec x8 * weights x32
HS = 8.0           # fact-GRU hidden scale (fp8 h states)
GSCF = 1.0 / (WS * HS)

HOST_PREP_VERSION = 6

_COMPILED = {}


class Cfg:
    def __init__(self, bc=BC, nf=NF, l=L, ql=QL, ep=EPISODES, nd=4, v=V,
                 fchunk=FCHUNK):
        self.bc, self.nf, self.l, self.ql, self.ep, self.nd, self.v = \
            bc, nf, l, ql, ep, nd, v
        self.s = bc * nf
        self.fchunk = min(fchunk, self.s)
        assert self.s % self.fchunk == 0
        self.nfc = self.s // self.fchunk
        self.vblks = [min(VBLK, v - i) for i in range(0, v, VBLK)]
        self.nv = bc * nd
        self.key = (bc, nf, l, ql, ep, nd, v, self.fchunk)


def _wt(wsb, k, m):
    """lhsT tile (128,128) of a weight sbuf tensor laid out (128, KT*G3)."""
    return wsb[:, k * G3 + m * 128:k * G3 + (m + 1) * 128]


def build(cfg: Cfg):
    nc = bacc.Bacc("TRN2", target_bir_lowering=False, debug=False,
                   num_devices=N_CORES)
    bc, nf, l, ql, ep, nd, v = cfg.bc, cfg.nf, cfg.l, cfg.ql, cfg.ep, cfg.nd, cfg.v
    s, ch, nfc, nv = cfg.s, cfg.fchunk, cfg.nfc, cfg.nv

    def din(name, shape, dt=BF16):
        return nc.dram_tensor(name, list(shape), dt, kind="ExternalInput").ap()

    gif_d = din("gif", (128, l - 1, nfc, MT * ch))
    h1f_d = din("h1f", (128, nfc, HQ * ch))
    giq_d = din("giq", (128, (ql - 1) * MT * bc))
    h1q_d = din("h1q", (128, HQ * bc))
    ident_d = din("ident", (128, 128))
    w_f_hh = din("w_f_hh", (128, HQ * G3), FP8)
    w_q_hh = din("w_q_hh", (128, HQ * G3), FP8)
    w_a_ih = din("w_a_ih", (128, HQ * G3), FP8)
    w_a_hh = din("w_a_hh", (128, HQ * G3), FP8)
    w_m_ih = din("w_m_ih", (128, HQ * G3), FP8)
    w_m_hh = din("w_m_hh", (128, HQ * G3), FP8)
    w_ans_ih = din("w_ans_ih", (128, 2 * HQ * G3), FP8)
    w_ans_hh = din("w_ans_hh", (128, HQ * G3), FP8)
    g1t = din("g1t", (128, 16 * H))
    g2t_d = din("g2t", (128, HQ))
    fct = din("fct", (128, HQ, v), FP8)
    fcb = din("fcb", (1, v))
    y0t_d = din("y0t", (128, HQ))
    gib_a_d = din("gib_a", (128, MT), F32)
    gib_ans_d = din("gib_ans", (128, MT), F32)
    bnhh_f_d = din("bnhh_f", (128, 4), F32)
    bnhh_q_d = din("bnhh_q", (128, 4), F32)
    bnhh_a_d = din("bnhh_a", (128, 4), F32)
    bnhh_ans_d = din("bnhh_ans", (128, 4), F32)
    brz_m_d = din("brz_m", (128, 8), F32)
    bnih_m_d = din("bnih_m", (128, 4), F32)
    bnhh_m_d = din("bnhh_m", (128, 4), F32)
    gb1_d = din("gb1", (128, HQ), F32)
    gb2_d = din("gb2", (1, 1), F32)
    out_d = nc.dram_tensor("out", [nv, v], F32, kind="ExternalOutput").ap()

    with tile.TileContext(nc) as tc, tc.tile_pool(name="const", bufs=1) as cp:
        frepT = cp.tile([128, HQ * s], BF16, tag="frepT")
        qrepT = cp.tile([128, HQ * bc], BF16, tag="qrepT")
        memT = cp.tile([128, HQ * bc], BF16, tag="memT")
        ones_nv = cp.tile([1, nv], BF16, tag="ones_nv")
        nc.vector.memset(ones_nv[:], 1.0)
        ones_128 = cp.tile([1, 128], BF16, tag="ones_128")
        nc.vector.memset(ones_128[:], 1.0)

        def load(ap_d, shape, dt=F32):
            t = cp.tile(list(shape), dt, tag=ap_d.tensor.name + "_sb")
            nc.sync.dma_start(t[:], ap_d[:])
            return t

        gib_a = load(gib_a_d, (128, MT))
        gib_ans = load(gib_ans_d, (128, MT))
        bnhh_f = load(bnhh_f_d, (128, 4))
        ident = load(ident_d, (128, 128), BF16)
        gb1 = load(gb1_d, (128, HQ))
        gb2 = load(gb2_d, (1, 1))
        y0t = load(y0t_d, (128, HQ), BF16)
        g2t = load(g2t_d, (128, HQ), BF16)

        def bcast_cols(src, ncol, rep, tag, dt=F32):
            t = cp.tile([128, ncol * rep], dt, tag=tag)
            nc.vector.tensor_copy(
                t[:].rearrange("p (c r) -> p c r", c=ncol),
                src[:].to_broadcast([128, ncol, rep]))
            return t

        bnhhx_q = bcast_cols(load(bnhh_q_d, (128, 4)), 4, bc, "bnhhx_q", BF16)
        bnhhx_a = bcast_cols(load(bnhh_a_d, (128, 4)), 4, bc, "bnhhx_a", BF16)
        bnhhx_ans = bcast_cols(load(bnhh_ans_d, (128, 4)), 4, bc,
                               "bnhhx_ans", BF16)
        brzx_m = bcast_cols(load(brz_m_d, (128, 8)), 8, bc, "brzx_m", BF16)
        bnihx_m = bcast_cols(load(bnih_m_d, (128, 4)), 4, bc, "bnihx_m", BF16)
        bnhhx_m = bcast_cols(load(bnhh_m_d, (128, 4)), 4, bc, "bnhhx_m", BF16)

        def gru_step(sp, pp, hh, h_ap, out_ap, bn, girz, gin,
                     bnhhx16, nih=None, bnihx16=None, wg_ap=None):
            """One folded GRU step (bn<=32), input gates pre-accumulated.

            hh: [(wsb, src, nk), ...] matmul contributions to the rz slabs;
            hh[0] also feeds the n_hh slab. girz: (128, 8bn) bf16 input-gates
            (+biases) added into PSUM via identity-matmul (no h dependency, so
            it lands before the recurrent matmuls). gin: (128, 4bn) n-gate
            input part (SBUF, added on DVE) or None with nih=(wsb, src, nk)
            for a separate n_ih PSUM slab (+bnihx16 identity). bnhhx16 is
            identity-added into the n_hh slab. wg_ap: per-step gate g; out
            becomes h + g*(1-z)*(n-h).
            """
            ps_r = pp.tile([128, 4 * bn], F32, tag="gpsr")
            ps_z = pp.tile([128, 4 * bn], F32, tag="gpsz")
            ps_n = pp.tile([128, 4 * bn], F32, tag="gpsn")
            nc.tensor.matmul(ps_r[:], ident[:], girz[:, 0:4 * bn],
                             start=True, stop=False)
            nc.tensor.matmul(ps_z[:], ident[:], girz[:, 4 * bn:8 * bn],
                             start=True, stop=False)
            nc.tensor.matmul(ps_n[:], ident[:], bnhhx16[:],
                             start=True, stop=False)
            if nih is not None:
                ps_ni = pp.tile([128, 4 * bn], F32, tag="gpsni")
                nc.tensor.matmul(ps_ni[:], ident[:], bnihx16[:],
                                 start=True, stop=False)
            # slab order r, n, z: sig_r overlaps the n matmuls and t1's
            # ps_n dependency resolves before the z slabs finish streaming
            ncon = sum(c[2] for c in hh)
            for m in range(4):
                i = 0
                for (wsb, src, nk) in hh:
                    for k in range(nk):
                        i += 1
                        nc.tensor.matmul(ps_r[:, m * bn:(m + 1) * bn],
                                         _wt(wsb, k, m),
                                         src[:, k * bn:(k + 1) * bn],
                                         start=False, stop=i == ncon)
            wsb0, src0, nk0 = hh[0]
            for j in range(4):
                for k in range(nk0):
                    nc.tensor.matmul(ps_n[:, j * bn:(j + 1) * bn],
                                     _wt(wsb0, k, 8 + j),
                                     src0[:, k * bn:(k + 1) * bn],
                                     start=False, stop=k == nk0 - 1)
            for m in range(4, 8):
                i = 0
                for (wsb, src, nk) in hh:
                    for k in range(nk):
                        i += 1
                        nc.tensor.matmul(ps_z[:, (m - 4) * bn:(m - 3) * bn],
                                         _wt(wsb, k, m),
                                         src[:, k * bn:(k + 1) * bn],
                                         start=False, stop=i == ncon)
            if nih is not None:
                wsb1, src1, nk1 = nih
                for j in range(4):
                    for k in range(nk1):
                        nc.tensor.matmul(ps_ni[:, j * bn:(j + 1) * bn],
                                         _wt(wsb1, k, 8 + j),
                                         src1[:, k * bn:(k + 1) * bn],
                                         start=False, stop=k == nk1 - 1)

            r_sb = sp.tile([128, 4 * bn], BF16, tag="rsb")
            z_sb = sp.tile([128, 4 * bn], BF16, tag="zsb")
            nc.scalar.activation(r_sb[:], ps_r[:], AF.Sigmoid, scale=GSC)
            nc.scalar.activation(z_sb[:], ps_z[:], AF.Sigmoid, scale=GSC)
            t1 = sp.tile([128, 4 * bn], BF16, tag="t1s")
            nc.vector.tensor_mul(t1[:], ps_n[:], r_sb[:])
            npre = sp.tile([128, 4 * bn], BF16, tag="npres")
            if nih is None:
                nc.vector.tensor_add(npre[:], t1[:], gin)
            else:
                nc.vector.tensor_add(npre[:], t1[:], ps_ni[:])
            # effective update weight w = (1-z) (optionally * g); overlaps tanh
            w2 = sp.tile([128, 4 * bn], BF16, tag="w2s")
            nc.vector.tensor_scalar(w2[:], z_sb[:], -1.0, 1.0,
                                    ALU.mult, ALU.add)
            if wg_ap is not None:
                wg = sp.tile([128, 4 * bn], BF16, tag="wgs")
                nc.vector.tensor_mul(
                    wg[:].rearrange("p (q b) -> p q b", q=HQ),
                    w2[:].rearrange("p (q b) -> p q b", q=HQ), wg_ap)
                w2 = wg
            n_t = sp.tile([128, 4 * bn], BF16, tag="ns")
            nc.scalar.activation(n_t[:], npre[:], AF.Tanh, scale=GSC)
            # h' = h + w * (n - h)
            d_t = sp.tile([128, 4 * bn], BF16, tag="ds")
            nc.vector.tensor_sub(d_t[:], n_t[:], h_ap)
            u_t = sp.tile([128, 4 * bn], BF16, tag="us")
            nc.vector.tensor_mul(u_t[:], w2[:], d_t[:])
            nc.vector.tensor_add(out_ap, h_ap, u_t[:])

        # -------- facts GRU with question GRU interleaved --------
        # Input-gate path is a host-precomputed per-token gather (gi tables,
        # biases folded); step 0 is folded into a host-gathered h1 table.
        with tc.tile_pool(name="wf", bufs=1) as wf, \
             tc.tile_pool(name="fxp", bufs=6) as xp, \
             tc.tile_pool(name="fps", bufs=5, space="PSUM") as pp, \
             tc.tile_pool(name="qpsB", bufs=1, space="PSUM") as ppb, \
             tc.tile_pool(name="fsp", bufs=3) as sp, \
             tc.tile_pool(name="fst", bufs=1) as stp:
            whh = wf.tile([128, HQ * G3], FP8, tag="whh")
            nc.sync.dma_start(whh[:], w_f_hh[:])
            whhq = wf.tile([128, HQ * G3], FP8, tag="whhq")
            nc.sync.dma_start(whhq[:], w_q_hh[:])
            giq = stp.tile([128, (ql - 1) * MT * bc], BF16, tag="giq")
            nc.sync.dma_start(giq[:], giq_d[:])
            hq = stp.tile([128, HQ * bc], BF16, tag="hq")
            nc.sync.dma_start(hq[:], h1q_d[:])
            qstep = [1]

            def q_step():
                t = qstep[0]
                if t >= ql:
                    return
                qstep[0] += 1
                out_ap = qrepT[:] if t == ql - 1 else hq[:]
                gt = giq[:, (t - 1) * MT * bc:t * MT * bc]
                gru_step(sp, ppb, [(whhq, hq[:], HQ)], hq[:], out_ap, bc,
                         girz=gt[:, 0:8 * bc], gin=gt[:, 8 * bc:12 * bc],
                         bnhhx16=bnhhx_q)

            hst = []
            for c in range(nfc):
                h = stp.tile([128, HQ * ch], BF16, tag=f"hf{c}")
                nc.sync.dma_start(h[:], h1f_d[:, c])
                hst.append(h)
            for t in range(1, l):
                for c in range(nfc):
                    h = hst[c]
                    gt = xp.tile([128, MT * ch], BF16, tag="gt")
                    nc.sync.dma_start(gt[:], gif_d[:, t - 1, c])
                    rz = sp.tile([128, 8 * ch], BF16, tag="rzf")
                    for m in range(8):
                        psm = pp.tile([128, ch], F32, tag="fg")
                        nc.tensor.matmul(psm[:], ident[:],
                                         gt[:, m * ch:(m + 1) * ch],
                                         start=True, stop=False)
                        for k in range(HQ):
                            nc.tensor.matmul(psm[:], _wt(whh, k, m),
                                             h[:, k * ch:(k + 1) * ch],
                                             start=False, stop=k == HQ - 1)
                        nc.scalar.activation(rz[:, m * ch:(m + 1) * ch], psm[:],
                                             AF.Sigmoid, scale=GSC)
                    t_sb = sp.tile([128, 4 * ch], BF16, tag="tf")
                    for j in range(4):
                        psm = pp.tile([128, ch], F32, tag="fg")
                        for k in range(HQ):
                            nc.tensor.matmul(psm[:], _wt(whh, k, 8 + j),
                                             h[:, k * ch:(k + 1) * ch],
                                             start=k == 0, stop=k == HQ - 1)
                        nc.vector.scalar_tensor_tensor(
                            t_sb[:, j * ch:(j + 1) * ch], psm[:],
                            bnhh_f[:, j:j + 1], rz[:, j * ch:(j + 1) * ch],
                            ALU.add, ALU.mult)
                    npre = sp.tile([128, 4 * ch], F32, tag="npf")
                    nc.vector.tensor_add(npre[:], t_sb[:],
                                         gt[:, 8 * ch:12 * ch])
                    n_t = sp.tile([128, 4 * ch], BF16, tag="nf")
                    nc.scalar.activation(n_t[:], npre[:], AF.Tanh, scale=GSC)
                    # h' = n + z * (h - n)
                    d_t = sp.tile([128, 4 * ch], BF16, tag="df")
                    nc.vector.tensor_sub(d_t[:], h[:], n_t[:])
                    u_t = sp.tile([128, 4 * ch], BF16, tag="uf")
                    nc.vector.tensor_mul(u_t[:], rz[:, 4 * ch:8 * ch], d_t[:])
                    if t == l - 1:
                        out_ap = frepT[:].rearrange(
                            "p (q sq) -> p q sq", q=HQ)[:, :, c * ch:(c + 1) * ch]
                        nc.vector.tensor_add(
                            out_ap, n_t[:].rearrange("p (q sq) -> p q sq", q=HQ),
                            u_t[:].rearrange("p (q sq) -> p q sq", q=HQ))
                    else:
                        nc.vector.tensor_add(h[:], n_t[:], u_t[:])
                    q_step()
            while qstep[0] < ql:
                q_step()

        # ---------------- episodic memory ----------------
        with tc.tile_pool(name="we", bufs=1) as we, \
             tc.tile_pool(name="epsA", bufs=3, space="PSUM") as ppa, \
             tc.tile_pool(name="epsB", bufs=1, space="PSUM") as ppb, \
             tc.tile_pool(name="eps2", bufs=1, space="PSUM") as pp2, \
             tc.tile_pool(name="esp", bufs=3) as sp, \
             tc.tile_pool(name="est", bufs=1) as stp:
            wiha = we.tile([128, HQ * G3], FP8, tag="wiha")
            whha = we.tile([128, HQ * G3], FP8, tag="whha")
            wihm = we.tile([128, HQ * G3], FP8, tag="wihm")
            whhm = we.tile([128, HQ * G3], FP8, tag="whhm")
            g1sb = we.tile([128, 16 * H], BF16, tag="g1sb")
            nc.sync.dma_start(wiha[:], w_a_ih[:])
            nc.sync.dma_start(whha[:], w_a_hh[:])
            nc.sync.dma_start(wihm[:], w_m_ih[:])
            nc.sync.dma_start(whhm[:], w_m_hh[:])
            nc.sync.dma_start(g1sb[:], g1t[:])
            nc.vector.tensor_copy(memT[:], qrepT[:])
            qexp = stp.tile([128, HQ * s], BF16, tag="qexp")
            nc.vector.tensor_copy(
                qexp[:].rearrange("p (q f b) -> p q f b", q=HQ, f=nf),
                qrepT[:].rearrange("p (q b) -> p q () b", q=HQ)
                .to_broadcast([128, HQ, nf, bc]))
            zfeat = stp.tile([128, 16 * s], BF16, tag="zfeat")
            mexp = stp.tile([128, HQ * s], BF16, tag="mexp")
            gia = stp.tile([128, nf * MT * bc], BF16, tag="gia")   # [f, m, b]
            gex = stp.tile([128, s], BF16, tag="gex")
            he = stp.tile([128, HQ * bc], BF16, tag="he")
            sblk = [min(VBLK, s - i) for i in range(0, s, VBLK)]
            # episode-invariant: f*q, |f-q| zfeat halves and the attention
            # GRU's input gates gi_a = Wih_a @ frep (+bias)
            nc.vector.tensor_mul(zfeat[:, 0:HQ * s], frepT[:], qexp[:])
            t3 = sp.tile([128, HQ * s], F32, tag="zt")
            nc.vector.tensor_sub(t3[:], frepT[:], qexp[:])
            nc.scalar.activation(zfeat[:, 2 * HQ * s:3 * HQ * s], t3[:], AF.Abs)
            gia4 = gia[:].rearrange("p (f m b) -> p f m b", f=nf, m=MT)
            for m in range(MT):
                off = 0
                for nb in sblk:
                    psm = ppa.tile([128, VBLK], F32, tag="eg")
                    for k in range(HQ):
                        nc.tensor.matmul(
                            psm[:, 0:nb], _wt(wiha, k, m),
                            frepT[:, k * s + off:k * s + off + nb],
                            start=k == 0, stop=k == HQ - 1)
                    # seq is f-major: psum cols land at [f, m, b] directly
                    nc.scalar.activation(
                        gia4[:, off // bc:(off + nb) // bc, m, :],
                        psm[:, 0:nb].rearrange("p (f b) -> p f b", b=bc),
                        AF.Identity, bias=gib_a[:, m:m + 1])
                    off += nb
            for e in range(ep):
                nc.vector.tensor_copy(
                    mexp[:].rearrange("p (q f b) -> p q f b", q=HQ, f=nf),
                    memT[:].rearrange("p (q b) -> p q () b", q=HQ)
                    .to_broadcast([128, HQ, nf, bc]))
                nc.vector.tensor_mul(zfeat[:, HQ * s:2 * HQ * s], frepT[:],
                                     mexp[:])
                t4 = sp.tile([128, HQ * s], F32, tag="zt")
                nc.vector.tensor_sub(t4[:], frepT[:], mexp[:])
                nc.scalar.activation(zfeat[:, 3 * HQ * s:4 * HQ * s], t4[:],
                                     AF.Abs)
                relu = sp.tile([128, HQ * s], BF16, tag="relu")
                for m in range(HQ):
                    off = 0
                    for nb in sblk:
                        psm = ppa.tile([128, VBLK], F32, tag="eg")
                        for k in range(16):
                            nc.tensor.matmul(
                                psm[:, 0:nb],
                                g1sb[:, k * H + m * 128:k * H + (m + 1) * 128],
                                zfeat[:, k * s + off:k * s + off + nb],
                                start=k == 0, stop=k == 15)
                        nc.scalar.activation(
                            relu[:, m * s + off:m * s + off + nb],
                            psm[:, 0:nb], AF.Relu, bias=gb1[:, m:m + 1])
                        off += nb
                off = 0
                for nb in sblk:
                    psg = pp2.tile([1, VBLK], F32, tag="eg2")
                    for k in range(HQ):
                        nc.tensor.matmul(psg[0:1, 0:nb], g2t[:, k:k + 1],
                                         relu[:, k * s + off:k * s + off + nb],
                                         start=k == 0, stop=k == HQ - 1)
                    nc.scalar.activation(gex[0:1, off:off + nb], psg[0:1, 0:nb],
                                         AF.Sigmoid, bias=gb2[:])
                    off += nb
                off = 0
                for nb in sblk:
                    psb = ppa.tile([128, VBLK], F32, tag="eg")
                    nc.tensor.matmul(psb[:, 0:nb], ones_128[:],
                                     gex[0:1, off:off + nb], start=True,
                                     stop=True)
                    nc.vector.tensor_copy(gex[:, off:off + nb], psb[:, 0:nb])
                    off += nb
                nc.vector.memset(he[:], 0.0)
                for t in range(nf):
                    gt = gia[:, t * MT * bc:(t + 1) * MT * bc]
                    gw = gex[:, t * bc:(t + 1) * bc].rearrange(
                        "p b -> p () b").to_broadcast([128, HQ, bc])
                    gru_step(sp, ppb, [(whha, he[:], HQ)], he[:], he[:], bc,
                             girz=gt[:, 0:8 * bc], gin=gt[:, 8 * bc:12 * bc],
                             bnhhx16=bnhhx_a, wg_ap=gw)
                gru_step(sp, ppb, [(whhm, memT[:], HQ), (wihm, he[:], HQ)],
                         memT[:], memT[:], bc, girz=brzx_m[:], gin=None,
                         bnhhx16=bnhhx_m, nih=(wihm, he[:], HQ),
                         bnihx16=bnihx_m)

        # ---------------- answer + fc/log-softmax ----------------
        with tc.tile_pool(name="wa", bufs=1) as wa, \
             tc.tile_pool(name="apsA", bufs=1, space="PSUM") as ppa, \
             tc.tile_pool(name="apsB", bufs=1, space="PSUM") as ppb, \
             tc.tile_pool(name="fcps", bufs=3, space="PSUM") as fpp, \
             tc.tile_pool(name="asp", bufs=3) as sp, \
             tc.tile_pool(name="ast", bufs=1) as stp, \
             tc.tile_pool(name="fcw", bufs=20) as fcp, tc.tile_pool(name="fco", bufs=2) as fop:
            wihans = wa.tile([128, 2 * HQ * G3], FP8, tag="wihans")
            whhans = wa.tile([128, HQ * G3], FP8, tag="whhans")
            nc.sync.dma_start(wihans[:], w_ans_ih[:])
            nc.sync.dma_start(whhans[:], w_ans_hh[:])
            ansin = stp.tile([128, 2 * HQ * bc], BF16, tag="ansin")
            nc.vector.tensor_copy(
                ansin[:, 0:HQ * bc].rearrange("p (q b) -> p q b", q=HQ),
                y0t[:].to_broadcast([128, HQ, bc]))
            nc.vector.tensor_copy(ansin[:, HQ * bc:2 * HQ * bc], qrepT[:])
            gians = stp.tile([128, MT * bc], BF16, tag="gians")
            for m in range(MT):
                psm = ppa.tile([128, bc], F32, tag="ag")
                for k in range(2 * HQ):
                    nc.tensor.matmul(psm[:], _wt(wihans, k, m),
                                     ansin[:, k * bc:(k + 1) * bc],
                                     start=k == 0, stop=k == 2 * HQ - 1)
                nc.scalar.activation(gians[:, m * bc:(m + 1) * bc], psm[:],
                                     AF.Identity, bias=gib_ans[:, m:m + 1])
            hdecT = stp.tile([128, HQ * nv], BF16, tag="hdecT")
            hdec8 = stp.tile([128, HQ * nv], FP8, tag="hdec8")
            hans = stp.tile([128, HQ * bc], BF16, tag="hans")
            nc.vector.tensor_copy(hans[:], memT[:])
            hd4 = hdecT[:].rearrange("p (q b dd) -> p q b dd", q=HQ, b=bc)
            for d in range(nd):
                gru_step(sp, ppb, [(whhans, hans[:], HQ)], hans[:], hans[:],
                         bc, girz=gians[:, 0:8 * bc],
                         gin=gians[:, 8 * bc:12 * bc], bnhhx16=bnhhx_ans)
                nc.vector.tensor_copy(
                    hd4[:, :, :, d:d + 1],
                    hans[:].rearrange("p (q b) -> p q b",
                                      q=HQ).to_broadcast([128, HQ, bc, 1]))
            nc.vector.tensor_scalar(hdec8[:], hdecT[:], 8.0, None, ALU.mult)
            hq3 = hdec8[:].rearrange("p (q n) -> p q n", q=HQ)
            logits = stp.tile([nv, v], BF16, tag="logits")
            sums = stp.tile([nv, len(cfg.vblks)], F32, tag="sums")
            off = 0
            for bi, nb in enumerate(cfg.vblks):
                wtl = fcp.tile([128, HQ * VBLK], FP8, tag="fcwt")
                nc.sync.dma_start(
                    wtl[:, 0:HQ * nb].rearrange("p (q n) -> p q n", q=HQ),
                    fct[:, :, off:off + nb])
                fcbt = fcp.tile([1, VBLK], BF16, tag="fcbt")
                nc.sync.dma_start(fcbt[0:1, 0:nb], fcb[0:1, off:off + nb])
                psm = fpp.tile([nv, VBLK], F32, tag="fps")
                w3 = wtl[:, 0:HQ * nb].rearrange("p (q n) -> p q n", q=HQ)
                for k in range(HQ // 2):
                    nc.tensor.matmul(psm[:, 0:nb],
                                     hq3[:, 2 * k:2 * k + 2, :],
                                     w3[:, 2 * k:2 * k + 2, :],
                                     start=k == 0, stop=False,
                                     perf_mode=mybir.MatmulPerfMode.DoubleRow)
                nc.tensor.matmul(psm[:, 0:nb], ones_nv[:], fcbt[0:1, 0:nb],
                                 start=False, stop=True)
                ex = sp.tile([nv, VBLK], BF16, tag="ex")
                nc.scalar.activation(ex[:, 0:nb], psm[:, 0:nb], AF.Exp,
                                     scale=1.0 / FCS, accum_out=sums[:, bi:bi + 1])
                nc.vector.tensor_scalar(logits[:, off:off + nb], psm[:, 0:nb],
                                        1.0 / FCS, None, ALU.mult)
                off += nb
            ssum = stp.tile([nv, 1], F32, tag="ssum")
            nc.vector.reduce_sum(ssum[:], sums[:], axis=mybir.AxisListType.X)
            logz = stp.tile([nv, 1], F32, tag="logz")
            nc.scalar.activation(logz[:], ssum[:], AF.Ln)
            ochunk = 4000
            for o0 in range(0, v, ochunk):
                o1 = min(o0 + ochunk, v)
                outb = fop.tile([nv, ochunk], F32, tag="outb")
                nc.vector.tensor_scalar(outb[:, 0:o1 - o0], logits[:, o0:o1],
                                        logz[:], None, ALU.subtract)
                nc.sync.dma_start(out_d[:, o0:o1], outb[:, 0:o1 - o0])
    nc.compile()
    return nc


def _sigmoid(x):
    return 1.0 / (1.0 + np.exp(-x))


def _gi_and_h1(x_gi, bhh_n):
    """Given input-gates (N, 3H) with rz biases (bih+bhh) and n bias (bih)
    folded, return first-step hidden h1 (N, H) (from h0 = 0)."""
    r1 = _sigmoid(x_gi[:, 0:H])
    z1 = _sigmoid(x_gi[:, H:2 * H])
    n1 = np.tanh(x_gi[:, 2 * H:3 * H] + r1 * bhh_n[None, :])
    return (1.0 - z1) * n1


def host_prep(inputs, cfg: Cfg):
    bc, nf, l, ql, nd, v = cfg.bc, cfg.nf, cfg.l, cfg.ql, cfg.nd, cfg.v
    ch, nfc = cfg.fchunk, cfg.nfc
    emb = np.asarray(inputs["emb"], np.float32).copy()
    emb[0] = 0.0
    facts = np.asarray(inputs["facts"])
    questions = np.asarray(inputs["questions"])
    b = facts.shape[0]
    ncores = b // bc

    flens = (np.asarray(inputs["facts_mask"]).reshape(b * nf, l) == 0).sum(-1)
    qlens = (np.asarray(inputs["question_masks"]) == 0).sum(-1)
    assert (flens == l).all() and (qlens == ql).all(), \
        "kernel requires full-length sequences (masks all zero)"

    ii = {k: np.asarray(vv, np.float32) for k, vv in inputs.items()
          if k not in ("facts", "facts_mask", "questions", "question_masks",
                       "num_decode")}

    def wt_tiles(w, kt, dt=bf16, scale=1.0):
        wt = w.T.reshape(kt, 128, w.shape[0]).transpose(1, 0, 2) * scale
        return np.ascontiguousarray(wt).reshape(128, kt * w.shape[0]).astype(dt)

    def col_tiles(x, ncol):
        return np.ascontiguousarray(x.reshape(ncol, 128).T).astype(np.float32)

    def gi_bias(bi, bh):
        return np.concatenate([(bi + bh)[0:2 * H], bi[2 * H:3 * H]])

    shared = {}
    for nm, wih, whh in (("a", "a_Wih", "a_Whh"), ("m", "m_Wih", "m_Whh")):
        shared[f"w_{nm}_ih"] = wt_tiles(ii[wih], HQ, f8, WS)
        shared[f"w_{nm}_hh"] = wt_tiles(ii[whh], HQ, f8, WS)
    shared["w_f_hh"] = wt_tiles(ii["ig_Whh"], HQ, f8, WS)
    shared["w_q_hh"] = wt_tiles(ii["qg_Whh"], HQ, f8, WS)
    shared["w_ans_ih"] = wt_tiles(ii["ans_Wih"], 2 * HQ, f8, WS)
    shared["w_ans_hh"] = wt_tiles(ii["ans_Whh"], HQ, f8, WS)
    g1 = ii["g_w1"].T  # (4H, H)
    shared["g1t"] = np.ascontiguousarray(
        g1.reshape(16, 128, H).transpose(1, 0, 2)).reshape(128, 16 * H).astype(bf16)
    shared["g2t"] = col_tiles(ii["g_w2"][0], HQ).astype(bf16)
    fcw = ii["fc_w"][:v] * FCW
    shared["fct"] = np.ascontiguousarray(
        fcw.T.reshape(HQ, 128, v).transpose(1, 0, 2)).astype(f8)
    shared["fcb"] = (ii["fc_b"][:v] * FCS).reshape(1, v).astype(bf16)
    shared["y0t"] = col_tiles(emb[1], HQ).astype(bf16)
    shared["ident"] = np.eye(128, dtype=bf16)

    for nm, bih, bhh in (("f", "ig_bih", "ig_bhh"), ("q", "qg_bih", "qg_bhh"),
                         ("a", "a_bih", "a_bhh"), ("m", "m_bih", "m_bhh"),
                         ("ans", "ans_bih", "ans_bhh")):
        bi, bh = ii[bih], ii[bhh]
        if nm in ("a", "ans"):
            shared[f"gib_{nm}"] = col_tiles(gi_bias(bi, bh) * WS, MT)
        shared[f"bnhh_{nm}"] = col_tiles(bh[2 * H:3 * H] * WS, 4)
        if nm == "m":
            shared[f"brz_{nm}"] = col_tiles((bi + bh)[0:2 * H] * WS, 8)
            shared[f"bnih_{nm}"] = col_tiles(bi[2 * H:3 * H] * WS, 4)
    shared["gb1"] = col_tiles(ii["g_b1"], HQ)
    shared["gb2"] = ii["g_b2"].reshape(1, 1).astype(np.float32)

    # ---- fused embedding -> input-gate tables (fact + question GRUs) ----
    # table[tok] = Wih @ emb0[tok] + gi_bias  (rz: bih+bhh folded; n: bih)
    tab_f = (emb @ ii["ig_Wih"].T + gi_bias(ii["ig_bih"], ii["ig_bhh"]))
    tab_f[0] = gi_bias(ii["ig_bih"], ii["ig_bhh"])  # padding token -> x=0
    # fact gi gather, steps 1..l-1, laid out (128, l-1, nfc, MT, ch)
    gi_f = (tab_f * WS).astype(bf16)[facts[:, :, 1:]]  # (B, NF, l-1, 3H)
    h1_f = _gi_and_h1(tab_f[facts[:, :, 0]].reshape(-1, 3 * H),
                      ii["ig_bhh"][2 * H:]).astype(bf16)  # (B*NF, H)
    qtok = questions                                   # (B, QL)
    gi_qf = (emb[qtok.reshape(-1)] @ ii["qg_Wih"].T
             + gi_bias(ii["qg_bih"], ii["qg_bhh"])).reshape(b, ql, 3 * H)
    h1_q = _gi_and_h1(gi_qf[:, 0], ii["qg_bhh"][2 * H:]).astype(bf16)  # (B, H)

    in_maps = []
    for c in range(ncores):
        m = dict(shared)
        s = bc * nf
        # seq index is f-major: seq = f * bc + b
        gf = np.ascontiguousarray(
            gi_f[c * bc:(c + 1) * bc].transpose(1, 0, 2, 3)
        ).reshape(s, l - 1, MT, 128)
        # -> (128, l-1, nfc, MT, ch)
        m["gif"] = np.ascontiguousarray(
            gf.reshape(nfc, ch, l - 1, MT, 128).transpose(4, 2, 0, 3, 1)
        ).reshape(128, l - 1, nfc, MT * ch)
        hf = np.ascontiguousarray(
            h1_f[c * bc * nf:(c + 1) * bc * nf].reshape(bc, nf, H)
            .transpose(1, 0, 2)).reshape(s, H)
        m["h1f"] = np.ascontiguousarray(
            hf.reshape(nfc, ch, HQ, 128).transpose(3, 0, 2, 1)
        ).reshape(128, nfc, HQ * ch)
        gq = gi_qf[c * bc:(c + 1) * bc, 1:] * WS       # (bc, ql-1, 3H)
        m["giq"] = np.ascontiguousarray(
            gq.reshape(bc, ql - 1, MT, 128).transpose(3, 1, 2, 0)
        ).astype(bf16).reshape(128, (ql - 1) * MT * bc)
        hq = h1_q[c * bc:(c + 1) * bc]                 # (bc, H)
        m["h1q"] = np.ascontiguousarray(
            hq.reshape(bc, HQ, 128).transpose(2, 1, 0)).reshape(128, HQ * bc)
        in_maps.append(m)
    return in_maps


def kernel(**inputs):
    nd = int(np.asarray(inputs["num_decode"]))
    cfg = Cfg(nd=nd)
    if cfg.key not in _COMPILED:
        _COMPILED[cfg.key] = build(cfg)
    nc = _COMPILED[cfg.key]
    in_maps = host_prep(inputs, cfg)
    res = bass_utils.run_bass_kernel_spmd(nc, in_maps,
                                          core_ids=list(range(N_CORES)))
    out = np.concatenate([res.results[c]["out"] for c in range(N_CORES)], 0)
    return np.ascontiguousarray(out.astype(np.float32))

